# revision 1
# baseline (speedup 1.0000x reference)
"""CliffordLinear (Cl(3,0)) Trainium2 kernel.

Math: Cl(3,0) is isomorphic to the algebra of 2x2 complex matrices via the
Pauli-matrix representation phi(x) = sum_a x_a * (s1^b0 s2^b1 s3^b2).  The
reference computes out[b,o] = sum_i W[o,i] * X[b,i] (Clifford product per
channel pair), which maps to OutM[b,o] = sum_i phi(W[o,i]) @ phi(X[b,i]) --
a 2x2 complex matrix contraction.  Splitting by output column c and
expanding complex arithmetic into real matmuls gives, per c in {0,1}:

    OutRe_c[b,(o,r)] = XRe_c @ R - XIm_c @ I
    OutIm_c[b,(o,r)] = XRe_c @ I + XIm_c @ R

with R/I = Re/Im of phi(W)[r,m] as [(i,m) x (o,r)] 512x512 matrices.  That
is 17.2G real MACs total vs 34.4G for the naive blade expansion (2x fewer).
The blade <-> Pauli basis changes are 8-point +- butterflies: the input side
is folded into host-side shard prep; the output side runs on the DVE while
evicting PSUM.  Matmuls run in float32r (TF32-like, ~1.5e-4 rel err, full
PE rate; plain fp32 is 4x slower).

Sharding: data-parallel over batch (1024 rows/core); weights replicated.
Per-core HBM traffic: 8.4 MB x + 2.1 MB w in, 8.4 MB out.
"""

import sys

sys.path.insert(0, "/opt/trn_rl_repo")

import numpy as np

import concourse.bass as bass  # noqa: F401  (registers lowerings)
import concourse.mybir as mybir
import concourse.tile as tile
from concourse import bacc
from concourse.bass_utils import run_bass_kernel_spmd

N_CORES = 8
B, CIN, COUT, NB = 8192, 256, 256, 8
BS = B // N_CORES          # 1024 batch rows per core
K = CIN * 4                # 1024 contraction rows (both halves)
HK = K // 2                # 512: rows per Re/Im half
OUTW = COUT * NB           # 2048 output width (o major, blade minor)
KT = K // 128              # 8 k-tiles of the x operand
BT = BS // 128             # 8 b-tiles

_cached = {}


def _build_nc():
    fr = mybir.dt.float32r
    f32 = mybir.dt.float32
    nc = bacc.Bacc("TRN2", target_bir_lowering=False, debug=False,
                   num_devices=N_CORES)
    # x'[c] layout: [bt, p, k, b] so each per-partition row is 4 KiB contiguous
    xt0 = nc.dram_tensor("xt0", [BT, 128, KT * 128], f32, kind="ExternalInput")
    xt1 = nc.dram_tensor("xt1", [BT, 128, KT * 128], f32, kind="ExternalInput")
    # weight planes R|I stacked: [2, 512, 512] = [Re/Im, (i,m), (o,r)]
    wri = nc.dram_tensor("wri", [2, HK, HK], f32, kind="ExternalInput")
    out = nc.dram_tensor("out", [BS, OUTW], f32, kind="ExternalOutput")

    with tile.TileContext(nc) as tc:
        with tc.tile_pool(name="wpool", bufs=1) as wpool, \
             tc.tile_pool(name="xpool", bufs=4) as xpool, \
             tc.tile_pool(name="opool", bufs=3) as opool, \
             tc.tile_pool(name="pspool", bufs=2, space="PSUM") as pspool:
            # PE warmup: ramp the clock gate during the initial DMA wait so
            # real matmuls start at full speed.  Zeros in, result unused.
            warm_in = wpool.tile([128, 640], mybir.dt.bfloat16, tag="warm_in")
            nc.vector.memset(warm_in[:], 0.0)
            warm_ps = pspool.tile([128, 512], f32, tag="ps0")
            for _ in range(8):
                nc.tensor.matmul(warm_ps[:], warm_in[:, :128], warm_in[:, 128:640],
                                 start=True, stop=True)

            # Startup interleave: bt0's x0 arrives in two 256 KiB chunks
            # around the weight-plane DMAs, so the first matmuls begin
            # after ~1.5 us of DMA instead of after the full 3 MB preload.
            x1_pre = xpool.tile([128, KT * 128], fr, tag="x1")
            x0_chunks = []
            for h in range(2):
                x0ph = xpool.tile([128, 512], fr, tag=f"x0p{h}", bufs=1)
                x0_chunks.append(x0ph)
            nc.sync.dma_start(x0_chunks[0][:], xt0[0][:, 0:512].bitcast(fr))
            r_t, i_t, ni_t = [], [], []
            for k in range(4):
                ik = wpool.tile([128, HK], fr, tag=f"i{k}")
                nc.sync.dma_start(ik[:], wri[1, k * 128:(k + 1) * 128, :].bitcast(fr))
                rk = wpool.tile([128, HK], fr, tag=f"r{k}")
                nc.sync.dma_start(rk[:], wri[0, k * 128:(k + 1) * 128, :].bitcast(fr))
                nik = wpool.tile([128, HK], fr, tag=f"ni{k}")
                nc.scalar.mul(nik[:], ik[:].bitcast(f32), -1.0)
                r_t.append(rk); i_t.append(ik); ni_t.append(nik)
            # x0's second chunk is first needed at k=4, after all w-planes
            nc.sync.dma_start(x0_chunks[1][:], xt0[0][:, 512:1024].bitcast(fr))
            nc.sync.dma_start(x1_pre[:], xt1[0].bitcast(fr))
            # rhs per (half, k): Re half: [R0..R3, -I0..-I3]; Im: [I0..I3, R0..R3]
            rhs_re = r_t + ni_t
            rhs_im = i_t + r_t

            for bt in range(BT):
                if bt == 0:
                    x0_lhs = [x0_chunks[k // 4][:, (k % 4) * 128:(k % 4 + 1) * 128]
                              for k in range(KT)]
                    x1_s = x1_pre
                else:
                    x0_s = xpool.tile([128, KT * 128], fr, tag="x0")
                    x1_s = xpool.tile([128, KT * 128], fr, tag="x1")
                    nc.sync.dma_start(x0_s[:], xt0[bt].bitcast(fr))
                    nc.sync.dma_start(x1_s[:], xt1[bt].bitcast(fr))
                    x0_lhs = [x0_s[:, k * 128:(k + 1) * 128] for k in range(KT)]
                x1_lhs = [x1_s[:, k * 128:(k + 1) * 128] for k in range(KT)]
                ps0 = pspool.tile([128, K], f32, tag="ps0")
                ps1 = pspool.tile([128, K], f32, tag="ps1")
                last = bt == BT - 1
                if not last:
                    for xlhs, ps in ((x0_lhs, ps0), (x1_lhs, ps1)):
                        for k in range(KT):
                            # Im first: its rhs never depends on the ScalarE
                            # negation, so a late nI_k can't stall it in the
                            # PE queue.
                            nc.tensor.matmul(ps[:, HK:K], xlhs[k], rhs_im[k][:],
                                             start=(k == 0), stop=(k == KT - 1))
                            nc.tensor.matmul(ps[:, 0:HK], xlhs[k], rhs_re[k][:],
                                             start=(k == 0), stop=(k == KT - 1))
                else:
                    # c1 first (so its eviction overlaps c0), and c0 split in
                    # two column chunks with separate PSUM tiles so chunk A's
                    # butterfly+store overlap chunk B's matmuls.
                    for k in range(KT):
                        nc.tensor.matmul(ps1[:, HK:K], x1_lhs[k], rhs_im[k][:],
                                         start=(k == 0), stop=(k == KT - 1))
                        nc.tensor.matmul(ps1[:, 0:HK], x1_lhs[k], rhs_re[k][:],
                                         start=(k == 0), stop=(k == KT - 1))
                    ps0a = ps0  # reuse the already-allocated ps0 slot: chunk A
                    ps0b = pspool.tile([128, K], f32, tag="ps1")
                    # Re chunk in bank 0, Im chunk in bank 1 (interleaved
                    # accumulation groups must not share a PSUM bank)
                    for cs, pst in ((0, ps0a), (1, ps0b)):
                        for k in range(KT):
                            nc.tensor.matmul(
                                pst[:, 0:256], x0_lhs[k],
                                rhs_re[k][:, cs * 256:(cs + 1) * 256],
                                start=(k == 0), stop=(k == KT - 1))
                            nc.tensor.matmul(
                                pst[:, HK:HK + 256], x0_lhs[k],
                                rhs_im[k][:, cs * 256:(cs + 1) * 256],
                                start=(k == 0), stop=(k == KT - 1))
                stage = opool.tile([128, OUTW], f32, tag="stage")
                # DVE reads only one PSUM operand: evict ps1 via ScalarE
                s1 = opool.tile([128, K], f32, tag="s1")
                nc.scalar.copy(s1[:], ps1[:])
                # inverse Pauli butterfly into blade-minor layout.
                # ps cols: [Re(o,r) | Im(o,r)], (o,r) packed o*2+r.
                # A=P00 (ps0,r0)  C=P10 (ps0,r1)  B=P01 (ps1,r0)  D=P11 (ps1,r1)
                # 4 dual-blade ops via 2-dim free APs (j picks Re/Im half):
                #   add (x0,x7): out 8o+7j      = ps0[512j+2o]   + s1[512j+2o+1]
                #   sub (x4,x3): out 8o+4-j     = ps0[512j+2o]   - s1[512j+2o+1]
                #   add (x1,x6): out 8o+1+5j    = ps0[512j+2o+1] + s1[512j+2o]
                #   sub (x5,x2): out 8o+5-3j    = ps0[512j+2o+1] - s1[512j+2o]
                def _ap3(base, off, jstep, ostep, ocnt):
                    a = base.copy()
                    part = a.ap.to_list()[0]
                    v = a.ap
                    v.clear()
                    v.extend([tuple(part), (jstep, 2), (ostep, ocnt)])
                    a.offset = a.offset + off
                    return a
                add, sub = nc.vector.tensor_add, nc.vector.tensor_sub
                if not last:
                    chunks = [(ps0, 0, HK, 0, 256, nc.sync)]
                else:
                    chunks = [(ps0a, 0, HK, 0, 128, nc.sync),
                              (ps0b, 0, HK, 256, 128, nc.scalar)]
                for pst, po, pjstep, so1, ocnt, dma_eng in chunks:
                    so = so1 * 4              # stage column offset of chunk
                    add(_ap3(stage[:], so + 0, 7, 8, ocnt),
                        _ap3(pst[:], po + 0, pjstep, 2, ocnt),
                        _ap3(s1[:], so1 + 1, HK, 2, ocnt))
                    sub(_ap3(stage[:], so + 4, -1, 8, ocnt),
                        _ap3(pst[:], po + 0, pjstep, 2, ocnt),
                        _ap3(s1[:], so1 + 1, HK, 2, ocnt))
                    add(_ap3(stage[:], so + 1, 5, 8, ocnt),
                        _ap3(pst[:], po + 1, pjstep, 2, ocnt),
                        _ap3(s1[:], so1 + 0, HK, 2, ocnt))
                    sub(_ap3(stage[:], so + 5, -3, 8, ocnt),
                        _ap3(pst[:], po + 1, pjstep, 2, ocnt),
                        _ap3(s1[:], so1 + 0, HK, 2, ocnt))
                    if last and so1 == 256:
                        # tail-critical store: two queues in parallel
                        half = ocnt * 4
                        nc.scalar.dma_start(
                            out[bt * 128:(bt + 1) * 128, so:so + half],
                            stage[:, so:so + half])
                        nc.sync.dma_start(
                            out[bt * 128:(bt + 1) * 128, so + half:so + ocnt * 8],
                            stage[:, so + half:so + ocnt * 8])
                    else:
                        dma_eng.dma_start(
                            out[bt * 128:(bt + 1) * 128, so:so + ocnt * 8],
                            stage[:, so:so + ocnt * 8])
    nc.finalize()
    return nc


def _pauli_parts(v):
    """v[..., 8] -> c0, c1 of shape [..., 2(m), 2(reim)]: the c-th column
    (Re, Im) of phi(v) rows m.  phi entries: A=P00=(v0+v4)+i(v3+v7),
    B=P01=(v1-v5)+i(v6-v2), C=P10=(v1+v5)+i(v6+v2), D=P11=(v0-v4)+i(v7-v3)."""
    c0 = np.empty(v.shape[:-1] + (2, 2), dtype=v.dtype)
    c1 = np.empty_like(c0)
    v0, v1, v2, v3, v4, v5, v6, v7 = (v[..., a] for a in range(8))
    c0[..., 0, 0] = v0 + v4   # Re A
    c0[..., 0, 1] = v3 + v7   # Im A
    c0[..., 1, 0] = v1 + v5   # Re C
    c0[..., 1, 1] = v6 + v2   # Im C
    c1[..., 0, 0] = v1 - v5   # Re B
    c1[..., 0, 1] = v6 - v2   # Im B
    c1[..., 1, 0] = v0 - v4   # Re D
    c1[..., 1, 1] = v7 - v3   # Im D
    return c0, c1


def _prep_w(weight):
    """weight [COUT, CIN, 8] -> [2, 512, 512] stacked R|I planes of
    phi(W)[r,m] indexed [(i,m), (o,r)], with the 0.5 inverse factor folded."""
    w = weight.astype(np.float32)
    # _pauli_parts returns matrix COLUMNS: cw_m[o,i,r,:] = (Re, Im) of
    # phi(W[o,i])[r, m].
    cw0, cw1 = _pauli_parts(w)
    R = np.empty((CIN, 2, COUT, 2), np.float32)   # [(i,m),(o,r)]
    I = np.empty_like(R)
    for m, cm in ((0, cw0), (1, cw1)):
        for r in range(2):
            R[:, m, :, r] = 0.5 * cm[:, :, r, 0].T
            I[:, m, :, r] = 0.5 * cm[:, :, r, 1].T
    return np.ascontiguousarray(
        np.stack([R.reshape(HK, HK), I.reshape(HK, HK)], axis=0))


def _prep_x(x):
    """x [B, CIN, 8] -> per-core xt arrays [N_CORES][BT, 128, KT*128] for
    c=0 and c=1, in the [bt, p, k, b] DMA-friendly layout.  Contraction row
    kappa = half*512 + i*2 + m  (half = 0:Re, 1:Im)."""
    xf = x.astype(np.float32)
    c0, c1 = _pauli_parts(xf)          # [B, CIN, m, reim]
    outs = []
    for arr in (c0, c1):
        # kappa-major array [K, B]: a = i*2+m ; kappa = ri*512 + a
        kb = arr.transpose(3, 1, 2, 0).reshape(K, B)   # [ri, i, m, b] -> [K, B]
        # device layout [core, bt, p, k, b]; kappa = k*128 + p
        a = kb.reshape(KT, 128, N_CORES, BT, 128)       # [k, p, core, bt, b]
        a = a.transpose(2, 3, 1, 0, 4)                  # [core, bt, p, k, b]
        outs.append(np.ascontiguousarray(
            a.reshape(N_CORES, BT, 128, KT * 128)))
    return outs


def kernel(x, weight, bias, cayley):
    assert x.shape == (B, CIN, NB) and weight.shape == (COUT, CIN, NB)
    if "nc" not in _cached:
        _cached["nc"] = _build_nc()
    nc = _cached["nc"]

    xt0, xt1 = _prep_x(np.asarray(x))
    wri = _prep_w(np.asarray(weight))
    in_maps = [{"xt0": xt0[c], "xt1": xt1[c], "wri": wri} for c in range(N_CORES)]
    res = run_bass_kernel_spmd(nc, in_maps, core_ids=list(range(N_CORES)))
    out = np.concatenate([res.results[c]["out"] for c in range(N_CORES)], axis=0)
    out = out.reshape(B, COUT, NB) + np.asarray(bias, np.float32)[None]
    return out.astype(np.float32)



# revision 8
# speedup vs baseline: 1.2757x; 1.2757x over previous
"""CliffordLinear (Cl(3,0)) Trainium2 kernel.

Math: Cl(3,0) is isomorphic to the 2x2 complex matrices via the Pauli
representation phi.  The reference's per-channel Clifford contraction maps to
one complex matrix product  phi(Out)[:,c] = phi(W) @ phi(X)[:,c]  per output
column c in {0,1}, i.e. for each c the real [b x 512] panels XRe, XIm against
the real 512x512 planes R, I of phi(W):

    Re = XRe@R - XIm@I          Im = XRe@I + XIm@R

Gauss's 3-multiplication trick computes both from three products

    G1 = XRe@R   G2 = XIm@I   G3 = (XRe+XIm)@(R+I)
    Re = G1 - G2                Im = G3 - G1 - G2

which is 12 accumulation k-tiles per (batch-tile, c) instead of 16 -- a 25%
Tensor-engine saving (12.9G real MACs vs 17.2G for the 4-mult scheme, vs
34.4G naive blades).  The XRe+XIm panel and R+I plane are precomputed on the
host and shipped, so the trick costs no device arithmetic on the input side.

All operands move as bf16 (half the HBM traffic of fp32; matmuls run at the
same PE rate as fp32r).  Per (bt, c): PE accumulates G1/G2/G3 into three
one-bank PSUM tiles; ScalarE evicts each to bf16 SBUF; DVE does the Gauss
recombine and the inverse-Pauli blade butterfly entirely in packed bf16
(2x DVE rate), writing a blade-major stage tile that DMAs out as bf16.
Weight columns are r-major (col = r*256 + o) so every butterfly operand is
contiguous; the host unshuffles the blade-major output.

Sharding: data-parallel over batch (1024 rows/core); weights replicated.
Per-core HBM: 6.3 MB x + 1.6 MB w in, 4.2 MB out (~33 us at 360 B/ns),
under the ~41 us PE floor.
"""

import sys

sys.path.insert(0, "/opt/trn_rl_repo")

import numpy as np

import concourse.bass as bass  # noqa: F401  (registers lowerings)
import concourse.mybir as mybir
import concourse.tile as tile
from concourse import bacc
from concourse.bass_utils import run_bass_kernel_spmd

N_CORES = 8
B, CIN, COUT, NB = 8192, 256, 256, 8
BS = B // N_CORES          # 1024 batch rows per core
BT = BS // 128             # 8 b-tiles
KP = 2 * CIN               # 512 contraction rows per panel (i,m)
PKT = KP // 128            # 4 k-tiles per panel
XKT = 3 * PKT              # 12 x k-tiles per c (Re | Im | Sum panels)
OUTW = COUT * NB           # 2048 output cols (blade-major: col = blade*256+o)

_cached = {}


def _rw_ap(base, off, dims):
    """Clone `base` keeping its leading (partition/row) dim, replacing the
    free dims with `dims` [(step, num), ...] and adding `off` elements."""
    a = base.copy()
    part = a.ap.to_list()[0]
    v = a.ap
    v.clear()
    v.extend([tuple(part)] + [tuple(d) for d in dims])
    a.offset = a.offset + off
    return a


def _build_nc():
    bf = mybir.dt.bfloat16
    f32 = mybir.dt.float32
    nc = bacc.Bacc("TRN2", target_bir_lowering=False, debug=False,
                   num_devices=N_CORES)
    # x panels per phi-column c: [bt, p, kk*128 + b] with contraction row
    # kappa = k*128 + p inside panel kk//4 (0:XRe, 1:XIm, 2:XRe+XIm)
    xt = [nc.dram_tensor(f"xt{c}", [BT, 128, XKT * 128], bf,
                         kind="ExternalInput") for c in range(2)]
    # weight planes [R, I, R+I]: rows kappa=(i,m), cols r-major r*256+o
    wt = nc.dram_tensor("wt", [3, KP, 512], bf, kind="ExternalInput")
    out = nc.dram_tensor("out", [BS, OUTW], bf, kind="ExternalOutput")

    with tile.TileContext(nc) as tc:
        with tc.tile_pool(name="wpool", bufs=1) as wpool, \
             tc.tile_pool(name="xpool", bufs=3) as xpool, \
             tc.tile_pool(name="epool", bufs=2) as epool, \
             tc.tile_pool(name="pspool", bufs=1, space="PSUM") as pspool:
            # PE warmup on garbage-free zeros: ramps the PE p-state during the
            # initial DMA wait so real matmuls start at full clock.
            warm_in = wpool.tile([128, 640], bf, tag="warm_in")
            nc.gpsimd.memset(warm_in[:], 0.0)
            warm_ps = pspool.tile([128, 512], f32, tag="warm_ps")
            for _ in range(5):
                nc.tensor.matmul(warm_ps[:], warm_in[:, :128], warm_in[:, 128:640],
                                 start=True, stop=True)

            # Startup: interleave weight planes with bt0's x panels in PE
            # consumption order (bt0 runs G-major: G1 both c, then G2, G3).
            # Plane 0 loads as 4 k-tile DMAs so the first matmul starts
            # early; planes 1/2 as one whole-plane DMA each (fewer HWDGE
            # slots).  Whole-plane SBUF layout: [p_low, k*512 + col].
            w0_t = [None] * PKT
            w_pl = [None] * 3
            x0_p = [[None] * 3 for _ in range(2)]   # [c][panel] tiles for bt0

            def _w0_dma(k):
                w0_t[k] = wpool.tile([128, 512], bf, tag=f"w0{k}", name=f"w0{k}")
                nc.sync.dma_start(w0_t[k][:], wt[0, k * 128:(k + 1) * 128, :])

            def _wpl_dma(p):
                w_pl[p] = wpool.tile([128, PKT * 512], bf, tag=f"wpl{p}",
                                     name=f"wpl{p}")
                # src rows kappa = k*128 + p_low -> [p_low, k, col]
                src = wt[p].copy()
                spart = tuple(src.ap.to_list()[0])
                v = src.ap
                v.clear()
                v.extend([(spart[0], 128), (spart[0] * 128, PKT), (1, 512)])
                nc.sync.dma_start(_rw_ap(w_pl[p][:], 0, [(512, PKT), (1, 512)]),
                                  src)

            def wrhs(p, k):
                if p == 0:
                    return w0_t[k]
                return w_pl[p][:, k * 512:(k + 1) * 512]

            def _x0_dma(c, p):
                x0_p[c][p] = xpool.tile([128, 512], bf, tag=f"x0{c}{p}",
                                        bufs=1, name=f"x0{c}{p}")
                nc.sync.dma_start(x0_p[c][p][:],
                                  xt[c][0][:, p * 512:(p + 1) * 512])

            _w0_dma(0)
            _x0_dma(0, 0)
            _x0_dma(1, 0)
            for k in range(1, PKT):
                _w0_dma(k)
            _wpl_dma(1)
            _x0_dma(0, 1)
            _x0_dma(1, 1)
            _wpl_dma(2)
            _x0_dma(0, 2)
            _x0_dma(1, 2)

            for bt in range(BT):
                if bt == 0:
                    def xlhs(c, p, k):
                        return x0_p[c][p][:, k * 128:(k + 1) * 128]
                else:
                    x_c = []
                    for c in range(2):
                        t = xpool.tile([128, XKT * 128], bf, tag=f"x{c}", name=f"x{c}")
                        nc.sync.dma_start(t[:], xt[c][bt])
                        x_c.append(t)

                    def xlhs(c, p, k, _x=x_c):
                        kk = p * PKT + k
                        return _x[c][:, kk * 128:(kk + 1) * 128]

                last = bt == BT - 1
                # Steady bts: one full-width pass.  Last bt: two half passes
                # split by the matrix-row index r (chunk 0 = {c0:r0, c1:r1}
                # feeding blades 0,4,7,3; chunk 1 = {c0:r1, c1:r0} feeding
                # blades 1,5,6,2) so the tail eviction overlaps matmuls.
                passes = [None] if not last else [0, 1]
                for h in passes:
                    wdt = 512 if h is None else 256
                    Gs = [[None] * 3, [None] * 3]

                    def emit_mm(c, p, h=h, wdt=wdt, Gs=Gs, xlhs=xlhs):
                        # chunk h=0: c0 -> r0 (off 0), c1 -> r1 (off 256)
                        # chunk h=1: c0 -> r1 (off 256), c1 -> r0 (off 0)
                        coff = 0 if (h is None or c == h) else 256
                        g = pspool.tile([128, 512], f32, tag=f"g{c}{p}",
                                        name=f"g{c}{p}")
                        for k in range(PKT):
                            rhs = wrhs(p, k) if h is None else \
                                wrhs(p, k)[:, coff:coff + 256]
                            nc.tensor.matmul(g[:, 0:wdt], xlhs(c, p, k), rhs,
                                             start=(k == 0), stop=(k == PKT - 1))
                        Gs[c][p] = g

                    if bt == 0:
                        # G-major: match the startup DMA arrival order
                        for p in range(3):
                            for c in range(2):
                                emit_mm(c, p)
                    else:
                        for c in range(2):
                            for p in range(3):
                                emit_mm(c, p)

                    t_c = []
                    for c in range(2):
                        G = Gs[c]
                        # ScalarE evicts PSUM -> bf16 SBUF (DVE can read at
                        # most one PSUM operand, and bf16 doubles DVE rate).
                        gs = []
                        for p in range(3):
                            s = epool.tile([128, 512], bf, tag=f"gs{c}{p}", name=f"gs{c}{p}")
                            nc.scalar.copy(s[:, 0:wdt], G[p][:, 0:wdt])
                            gs.append(s)
                        # Gauss recombine on DVE: t = [Re | Im] (wdt each)
                        t = epool.tile([128, 1024], bf, tag=f"t{c}", name=f"t{c}")
                        u = epool.tile([128, 512], bf, tag=f"u{c}", name=f"u{c}")
                        nc.vector.tensor_sub(t[:, 0:wdt], gs[0][:, 0:wdt],
                                             gs[1][:, 0:wdt])
                        nc.vector.tensor_add(u[:, 0:wdt], gs[0][:, 0:wdt],
                                             gs[1][:, 0:wdt])
                        nc.vector.tensor_sub(t[:, wdt:2 * wdt], gs[2][:, 0:wdt],
                                             u[:, 0:wdt])
                        t_c.append(t)

                    # Inverse-Pauli blade butterfly, one dual-blade op per
                    # (sum, diff) pair; j picks the Re/Im halves of t:
                    #   x0 = ReA+ReD  x4 = ReA-ReD  x7 = ImA+ImD  x3 = ImA-ImD
                    #   x1 = ReC+ReB  x5 = ReC-ReB  x6 = ImC+ImB  x2 = ImC-ImB
                    # A/C = r0/r1 of c0;  B/D = r0/r1 of c1.
                    stage = epool.tile([128, OUTW], bf, tag="stage")
                    add, sub = nc.vector.tensor_add, nc.vector.tensor_sub
                    inner = (1, 256)
                    if h is None or h == 0:
                        # (x0,x7) and (x4,x3): A-part of c0 with D-part of c1
                        a_off = 0 if h is None else 0
                        d_off = 256 if h is None else 0
                        js = 512 if h is None else 256
                        add(_rw_ap(stage[:], 0 * 256, [(1792, 2), inner]),
                            _rw_ap(t_c[0][:], a_off, [(js, 2), inner]),
                            _rw_ap(t_c[1][:], d_off, [(js, 2), inner]))
                        sub(_rw_ap(stage[:], 4 * 256, [(-256, 2), inner]),
                            _rw_ap(t_c[0][:], a_off, [(js, 2), inner]),
                            _rw_ap(t_c[1][:], d_off, [(js, 2), inner]))
                    if h is None or h == 1:
                        # (x1,x6) and (x5,x2): C-part of c0 with B-part of c1
                        c_off = 256 if h is None else 0
                        b_off = 0 if h is None else 0
                        js = 512 if h is None else 256
                        add(_rw_ap(stage[:], 1 * 256, [(1280, 2), inner]),
                            _rw_ap(t_c[0][:], c_off, [(js, 2), inner]),
                            _rw_ap(t_c[1][:], b_off, [(js, 2), inner]))
                        sub(_rw_ap(stage[:], 5 * 256, [(-768, 2), inner]),
                            _rw_ap(t_c[0][:], c_off, [(js, 2), inner]),
                            _rw_ap(t_c[1][:], b_off, [(js, 2), inner]))

                    # Steady stores go through gpsimd's SWDGE queue: its sem
                    # wait sits on the otherwise-idle Pool SEQ, so the SP
                    # load queue never stalls behind a store.  The two final
                    # stores use ACT+SP HWDGE queues (lower latency, and by
                    # then both queues are drained).
                    orows = out[bt * 128:(bt + 1) * 128, 0:OUTW]
                    if h is None:
                        nc.gpsimd.dma_start(orows, stage[:])
                    elif h == 0:
                        # blades 0,3 | 4,7 -> col blocks {0,768} and {1024,1792}
                        nc.gpsimd.dma_start(
                            _rw_ap(orows, 0, [(768, 2), inner]),
                            _rw_ap(stage[:], 0, [(768, 2), inner]))
                        nc.gpsimd.dma_start(
                            _rw_ap(orows, 1024, [(768, 2), inner]),
                            _rw_ap(stage[:], 1024, [(768, 2), inner]))
                    else:
                        # blades 1,5 | 2,6 -> {256,1280} and {512,1536}
                        nc.scalar.dma_start(
                            _rw_ap(orows, 256, [(1024, 2), inner]),
                            _rw_ap(stage[:], 256, [(1024, 2), inner]))
                        nc.sync.dma_start(
                            _rw_ap(orows, 512, [(1024, 2), inner]),
                            _rw_ap(stage[:], 512, [(1024, 2), inner]))
    nc.finalize()
    return nc


def _pauli_parts(v):
    """v[..., 8] -> c0, c1 of shape [..., 2(m/r), 2(reim)]: the c-th column
    (Re, Im) of phi(v).  phi entries: A=P00=(v0+v4)+i(v3+v7),
    B=P01=(v1-v5)+i(v6-v2), C=P10=(v1+v5)+i(v6+v2), D=P11=(v0-v4)+i(v7-v3)."""
    c0 = np.empty(v.shape[:-1] + (2, 2), dtype=v.dtype)
    c1 = np.empty_like(c0)
    v0, v1, v2, v3, v4, v5, v6, v7 = (v[..., a] for a in range(8))
    c0[..., 0, 0] = v0 + v4   # Re A
    c0[..., 0, 1] = v3 + v7   # Im A
    c0[..., 1, 0] = v1 + v5   # Re C
    c0[..., 1, 1] = v6 + v2   # Im C
    c1[..., 0, 0] = v1 - v5   # Re B
    c1[..., 0, 1] = v6 - v2   # Im B
    c1[..., 1, 0] = v0 - v4   # Re D
    c1[..., 1, 1] = v7 - v3   # Im D
    return c0, c1


def _np_bf16():
    return mybir.dt.np(mybir.dt.bfloat16)


def _prep_w(weight):
    """weight [COUT, CIN, 8] -> [3, 512, 512] planes [R, I, R+I] of
    phi(W)[r,m], rows (i,m), cols r-major (col = r*256 + o), 0.5 folded."""
    w = weight.astype(np.float32)
    cw0, cw1 = _pauli_parts(w)    # cw_m[o, i, r, (re,im)] = phi(W[o,i])[r,m]
    R = np.empty((CIN, 2, 2, COUT), np.float32)   # [(i,m),(r,o)]
    I = np.empty_like(R)
    for m, cm in ((0, cw0), (1, cw1)):
        for r in range(2):
            R[:, m, r, :] = 0.5 * cm[:, :, r, 0].T
            I[:, m, r, :] = 0.5 * cm[:, :, r, 1].T
    Rm = R.reshape(KP, 512)
    Im_ = I.reshape(KP, 512)
    return np.ascontiguousarray(
        np.stack([Rm, Im_, Rm + Im_], axis=0)).astype(_np_bf16())


def _prep_x(x):
    """x [B, CIN, 8] -> per-core arrays [N_CORES][BT, 128, XKT*128] bf16 for
    c in {0,1}: panels [XRe | XIm | XRe+XIm], device layout [bt, p, kk, b]
    with kappa = k*128 + p, col = kk*128 + b."""
    xf = x.astype(np.float32)
    c0, c1 = _pauli_parts(xf)          # [B, CIN, m, reim]
    outs = []
    for arr in (c0, c1):
        re = arr[..., 0].reshape(B, KP)          # kappa = i*2+m
        im = arr[..., 1].reshape(B, KP)
        panels = np.concatenate([re, im, re + im], axis=1)   # col = kk*128+p
        a = panels.reshape(N_CORES, BT, 128, XKT, 128)  # [core, bt, b, kk, p]
        a = a.transpose(0, 1, 4, 3, 2)                  # [core, bt, p, kk, b]
        outs.append(np.ascontiguousarray(
            a.reshape(N_CORES, BT, 128, XKT * 128)).astype(_np_bf16()))
    return outs


def kernel(x, weight, bias, cayley):
    assert x.shape == (B, CIN, NB) and weight.shape == (COUT, CIN, NB)
    if "nc" not in _cached:
        _cached["nc"] = _build_nc()
    nc = _cached["nc"]

    xp = _prep_x(np.asarray(x))
    wp = _prep_w(np.asarray(weight))
    in_maps = [{"xt0": xp[0][c], "xt1": xp[1][c], "wt": wp}
               for c in range(N_CORES)]
    res = run_bass_kernel_spmd(nc, in_maps, core_ids=list(range(N_CORES)))
    out = np.concatenate(
        [np.asarray(res.results[c]["out"]).astype(np.float32)
         for c in range(N_CORES)], axis=0)
    # cols are blade-major (blade*256 + o) -> [B, COUT, NB]
    out = out.reshape(B, NB, COUT).transpose(0, 2, 1)
    out = out + np.asarray(bias, np.float32)[None]
    return np.ascontiguousarray(out.astype(np.float32))


# revision 34
# speedup vs baseline: 1.3461x; 1.0552x over previous
"""CliffordLinear (Cl(3,0)) Trainium2 kernel.

Math: Cl(3,0) is isomorphic to the 2x2 complex matrices via the Pauli
representation phi.  The reference's per-channel Clifford contraction maps to
one complex matrix product  phi(Out)[:,c] = phi(W) @ phi(X)[:,c]  per output
column c in {0,1}, i.e. for each c the real [b x 512] panels XRe, XIm against
the real 512x512 planes R, I of phi(W):

    Re = XRe@R - XIm@I          Im = XRe@I + XIm@R

Gauss's 3-multiplication trick computes both from three products

    G1 = XRe@R   G2 = XIm@I   G3 = (XRe+XIm)@(R+I)
    Re = G1 - G2                Im = G3 - G1 - G2

which is 12 accumulation k-tiles per (batch-tile, c) instead of 16 -- a 25%
Tensor-engine saving (12.9G real MACs vs 17.2G for the 4-mult scheme, vs
34.4G naive blades).  The XRe+XIm panel and R+I plane are precomputed on the
host and shipped, so the trick costs no device arithmetic on the input side.

All operands move as bf16 (half the HBM traffic of fp32; matmuls run at the
same PE rate as fp32r).  Per (bt, c): PE accumulates G1/G2/G3 into three
one-bank PSUM tiles; ScalarE evicts each to bf16 SBUF; DVE does the Gauss
recombine and the inverse-Pauli blade butterfly entirely in packed bf16
(2x DVE rate), writing a blade-major stage tile that DMAs out as bf16.
Weight columns are r-major (col = r*256 + o) so every butterfly operand is
contiguous; the host unshuffles the blade-major output.

Sharding: data-parallel over batch (1024 rows/core); weights replicated.
Per-core HBM: 6.3 MB x + 1.6 MB w in, 4.2 MB out (~33 us at 360 B/ns),
under the ~41 us PE floor.
"""

import sys

sys.path.insert(0, "/opt/trn_rl_repo")

import numpy as np

import concourse.bass as bass  # noqa: F401  (registers lowerings)
import concourse.mybir as mybir
import concourse.tile as tile
from concourse import bacc
from concourse.bass_utils import run_bass_kernel_spmd

N_CORES = 8
B, CIN, COUT, NB = 8192, 256, 256, 8
BS = B // N_CORES          # 1024 batch rows per core
BT = BS // 128             # 8 b-tiles
KP = 2 * CIN               # 512 contraction rows per panel (i,m)
PKT = KP // 128            # 4 k-tiles per panel
XKT = 2 * PKT              # 8 x k-tiles per c (Re | Im; Sum panel is
                           # recomputed on-device, saving 1/3 of x HBM)
OUTW = COUT * NB           # 2048 output cols (blade-major: col = blade*256+o)

_cached = {}

N_WARM = 13        # PE p-state ramp matmuls before real data lands
PS_DB = 1          # ring depth for the first-per-c PSUM tags (g00/g10);
                   # 1 frees a bank for the split-G3c1 tail


def _rw_ap(base, off, dims):
    """Clone `base` keeping its leading (partition/row) dim, replacing the
    free dims with `dims` [(step, num), ...] and adding `off` elements."""
    a = base.copy()
    part = a.ap.to_list()[0]
    v = a.ap
    v.clear()
    v.extend([tuple(part)] + [tuple(d) for d in dims])
    a.offset = a.offset + off
    return a


def _build_nc():
    bf = mybir.dt.bfloat16
    f32 = mybir.dt.float32
    nc = bacc.Bacc("TRN2", target_bir_lowering=False, debug=False,
                   num_devices=N_CORES)
    # x panels per phi-column c: [bt, p, kk*128 + b] with contraction row
    # kappa = k*128 + p inside panel kk//4 (0:XRe, 1:XIm, 2:XRe+XIm)
    xt = [nc.dram_tensor(f"xt{c}", [BT, 128, XKT * 128], bf,
                         kind="ExternalInput") for c in range(2)]
    # weight planes [R, I]: rows kappa=(i,m), cols r-major r*256+o.
    # The Gauss R+I plane is summed on-device (saves startup HBM traffic).
    wt = nc.dram_tensor("wt", [2, KP, 512], bf, kind="ExternalInput")
    out = nc.dram_tensor("out", [BS, OUTW], bf, kind="ExternalOutput")

    with tile.TileContext(nc) as tc:
        with tc.tile_pool(name="wpool", bufs=1) as wpool, \
             tc.tile_pool(name="xpool", bufs=3) as xpool, \
             tc.tile_pool(name="epool", bufs=2) as epool, \
             tc.tile_pool(name="pspool", bufs=1, space="PSUM") as pspool:
            # PE warmup on zeros: ramps the PE p-state during the initial DMA
            # wait so real matmuls start at full clock.
            warm_in = wpool.tile([128, 384], bf, tag="warm_in")
            nc.vector.memset(warm_in[:].bitcast(mybir.dt.uint32), 0)
            # warm_ps shares the g00 ring (retired after warmup, so the
            # spare PSUM banks go to double-buffering g00/g10 instead)
            warm_ps = pspool.tile([128, 512], f32, tag="g00", bufs=PS_DB)
            for _ in range(N_WARM):
                nc.tensor.matmul(warm_ps[:, 0:256], warm_in[:, :128],
                                 warm_in[:, 128:384], start=True, stop=True)

            # Startup: interleave weight k-tiles with bt0's x panels in PE
            # consumption order (bt0 runs G-major: G1 both c, then G2, G3).
            # The R+I plane never moves over HBM -- DVE sums it from the R
            # and I tiles while the PE chews on G1/G2.
            w_t = [[None] * PKT for _ in range(3)]
            x0_p = [[None] * 3 for _ in range(2)]   # [c][panel] tiles for bt0

            def _w_kgrp(p, k0, nk):
                # one DMA covering k-tiles [k0, k0+nk) of plane p into
                # separate 512-col views of one tile: src rows kappa =
                # k*128 + p_low -> dest [p_low, k, col]
                t = wpool.tile([128, nk * 512], bf, tag=f"w{p}g{k0}",
                               name=f"w{p}g{k0}")
                for k in range(nk):
                    w_t[p][k0 + k] = t[:, k * 512:(k + 1) * 512]
                src = wt[p].copy()
                spart = tuple(src.ap.to_list()[0])
                v = src.ap
                v.clear()
                v.extend([(spart[0], 128), (spart[0] * 128, nk), (1, 512)])
                src.offset = src.offset + k0 * 128 * spart[0]
                nc.sync.dma_start(_rw_ap(t[:], 0, [(512, nk), (1, 512)]), src)

            def wrhs(p, k):
                return w_t[p][k]

            def _x0_dma(c, p):
                # bt0's x panels ride the gpsimd SWDGE queue: Pool generates
                # descriptors in parallel with HWDGE, halving the startup
                # per-DMA overhead serialization.
                x0_p[c][p] = xpool.tile([128, 512], bf, tag=f"x0{c}{p}",
                                        bufs=1, name=f"x0{c}{p}")
                nc.gpsimd.dma_start(x0_p[c][p][:],
                                    xt[c][0][:, p * 512:(p + 1) * 512])

            _x0_dma(0, 0)
            _w_kgrp(0, 0, 1)
            _x0_dma(1, 0)
            _w_kgrp(0, 1, 3)
            _x0_dma(0, 1)
            for k in range(PKT):
                _w_kgrp(1, k, 1)
            _x0_dma(1, 1)

            def _xsum(c, src_re, src_im):
                # on-device XSum panel for the G3 product
                xs = xpool.tile([128, 512], bf, tag=f"xs{c}", bufs=2,
                                name=f"xs{c}")
                nc.vector.tensor_add(xs[:], src_re, src_im)
                return xs

            xs0_c0 = _xsum(0, x0_p[0][0][:], x0_p[0][1][:])
            for k in range(PKT):
                w_t[2][k] = wpool.tile([128, 512], bf, tag=f"w2{k}",
                                       name=f"w2{k}")
                nc.vector.tensor_add(w_t[2][k][:], w_t[0][k], w_t[1][k])
            xs0_c1 = _xsum(1, x0_p[1][0][:], x0_p[1][1][:])

            def _mk_xlhs(panels, xs):
                # panels[c][p] for p in {0,1} are 512-col APs; xs[c] the sum
                def xlhs(c, p, k):
                    if p == 2:
                        return xs[c][:, k * 128:(k + 1) * 128]
                    return panels[c][p][:, k * 128:(k + 1) * 128]
                return xlhs

            cur_xlhs = _mk_xlhs(x0_p, [xs0_c0, xs0_c1])

            for bt in range(BT):
                # Prefetch bt+1's x one full window ahead: DMA + the
                # on-device XSum add, so neither is ever on the PE's path.
                if bt + 1 < BT:
                    nxt = []
                    for c in range(2):
                        t = xpool.tile([128, XKT * 128], bf, tag=f"x{c}",
                                       name=f"x{c}")
                        nc.sync.dma_start(t[:], xt[c][bt + 1])
                        nxt.append(t)
                    panels = [[t[:, 0:512], t[:, 512:1024]] for t in nxt]
                    xs = [_xsum(c, panels[c][0], panels[c][1])
                          for c in range(2)]
                    next_xlhs = _mk_xlhs(panels, xs)

                xlhs = cur_xlhs
                if bt + 1 < BT:
                    cur_xlhs = next_xlhs

                last = bt == BT - 1
                Gs = [[None] * 3, [None] * 3]

                def emit_mm(c, p, Gs=Gs, xlhs=xlhs):
                    # p=0 tags double-buffered: the next bt's first matmul
                    # group per c never waits on this bt's eviction copy
                    g = pspool.tile([128, 512], f32, tag=f"g{c}{p}",
                                    name=f"g{c}{p}", bufs=PS_DB if p == 0 else 1)
                    for k in range(PKT):
                        nc.tensor.matmul(g[:], xlhs(c, p, k), wrhs(p, k),
                                         start=(k == 0), stop=(k == PKT - 1))
                    Gs[c][p] = g

                if bt == 0 or last:
                    # G-major: bt0 matches the startup DMA arrival order; the
                    # last bt wants G1/G2 stopped early so the Re-blade
                    # eviction and stores run under the G3 matmuls.
                    for p in range(3):
                        for c in range(2):
                            emit_mm(c, p)
                else:
                    for c in range(2):
                        for p in range(3):
                            emit_mm(c, p)

                # Eviction.  ScalarE copies PSUM -> bf16 SBUF (DVE reads at
                # most one PSUM operand, and all-bf16 doubles the DVE rate);
                # DVE does the Gauss recombine into t = [Re 512 | Im 512]
                # (r-major halves: A/C = r0/r1 of c0, B/D = r0/r1 of c1) and
                # the inverse-Pauli butterfly into the blade-major stage:
                #   x0 = ReA+ReD  x4 = ReA-ReD  x7 = ImA+ImD  x3 = ImA-ImD
                #   x1 = ReC+ReB  x5 = ReC-ReB  x6 = ImC+ImB  x2 = ImC-ImB
                add, sub = nc.vector.tensor_add, nc.vector.tensor_sub
                inner = (1, 256)
                stage = epool.tile([128, OUTW], bf, tag="stage")
                orows = out[bt * 128:(bt + 1) * 128, 0:OUTW]
                # ACT copies in matmul-stop order so no copy head-of-line
                # blocks an already-stopped G behind it on the in-order ACT
                # engine (stops are G-major on bt0/last, c-major otherwise).
                t_c, u_c = [], []
                gs_c = [[None] * 3, [None] * 3]
                np_copy = 2 if last else 3
                order = [(c, p) for p in range(np_copy) for c in range(2)] \
                    if (bt == 0 or last) else \
                    [(c, p) for c in range(2) for p in range(np_copy)]
                for c, p in order:
                    s = epool.tile([128, 512], bf, tag=f"gs{c}{p}",
                                   name=f"gs{c}{p}")
                    nc.scalar.copy(s[:], Gs[c][p][:])
                    gs_c[c][p] = s
                for c in range(2):
                    gs = gs_c[c]
                    t = epool.tile([128, 1024], bf, tag=f"t{c}", name=f"t{c}")
                    u = epool.tile([128, 512], bf, tag=f"u{c}", name=f"u{c}")
                    nc.vector.tensor_sub(t[:, 0:512], gs[0][:], gs[1][:])
                    nc.vector.tensor_add(u[:], gs[0][:], gs[1][:])
                    if not last:
                        nc.vector.tensor_sub(t[:, 512:1024], gs[2][:], u[:])
                    t_c.append(t)
                    u_c.append(u)

                if not last:
                    # Dual-blade butterfly ops; j picks the Re/Im halves.
                    add(_rw_ap(stage[:], 0 * 256, [(1792, 2), inner]),
                        _rw_ap(t_c[0][:], 0, [(512, 2), inner]),
                        _rw_ap(t_c[1][:], 256, [(512, 2), inner]))
                    sub(_rw_ap(stage[:], 4 * 256, [(-256, 2), inner]),
                        _rw_ap(t_c[0][:], 0, [(512, 2), inner]),
                        _rw_ap(t_c[1][:], 256, [(512, 2), inner]))
                    add(_rw_ap(stage[:], 1 * 256, [(1280, 2), inner]),
                        _rw_ap(t_c[0][:], 256, [(512, 2), inner]),
                        _rw_ap(t_c[1][:], 0, [(512, 2), inner]))
                    sub(_rw_ap(stage[:], 5 * 256, [(-768, 2), inner]),
                        _rw_ap(t_c[0][:], 256, [(512, 2), inner]),
                        _rw_ap(t_c[1][:], 0, [(512, 2), inner]))
                    # Steady stores ride gpsimd's SWDGE queue: the sem wait
                    # parks on the otherwise-idle Pool SEQ, so the SP load
                    # queue never stalls behind a store.
                    nc.gpsimd.dma_start(orows, stage[:])
                else:
                    # Re/Im-phased tail: Re blades (j duals (x0,x1), (x4,x5))
                    # need only G1/G2 -- they evict and store while the G3
                    # matmuls still run.  Only the Im blades wait on G3.
                    add(_rw_ap(stage[:], 0, [(256, 2), inner]),
                        _rw_ap(t_c[0][:], 0, [(256, 2), inner]),
                        _rw_ap(t_c[1][:], 256, [(-256, 2), inner]))
                    sub(_rw_ap(stage[:], 1024, [(256, 2), inner]),
                        _rw_ap(t_c[0][:], 0, [(256, 2), inner]),
                        _rw_ap(t_c[1][:], 256, [(-256, 2), inner]))
                    nc.gpsimd.dma_start(orows[:, 0:512], stage[:, 0:512])
                    nc.gpsimd.dma_start(orows[:, 1024:1536], stage[:, 1024:1536])
                    # Im phase: both G3 copies on ACT; each overlaps DVE work
                    # on the other c, keeping the post-matmul chain short.
                    for c in range(2):
                        s = epool.tile([128, 512], bf, tag=f"gs{c}2",
                                       name=f"gs{c}2")
                        nc.scalar.copy(s[:], Gs[c][2][:])
                        gs_c[c][2] = s
                    nc.vector.tensor_sub(t_c[0][:, 512:1024], gs_c[0][2][:],
                                         u_c[0][:])
                    nc.vector.tensor_sub(t_c[1][:, 512:1024], gs_c[1][2][:],
                                         u_c[1][:])
                    sub(_rw_ap(stage[:], 768, [(-256, 2), inner]),
                        _rw_ap(t_c[0][:], 512, [(256, 2), inner]),
                        _rw_ap(t_c[1][:], 768, [(-256, 2), inner]))
                    nc.scalar.dma_start(orows[:, 512:1024], stage[:, 512:1024])
                    add(_rw_ap(stage[:], 1792, [(-256, 2), inner]),
                        _rw_ap(t_c[0][:], 512, [(256, 2), inner]),
                        _rw_ap(t_c[1][:], 768, [(-256, 2), inner]))
                    nc.sync.dma_start(orows[:, 1536:2048], stage[:, 1536:2048])
    nc.finalize()
    return nc


def _pauli_parts(v):
    """v[..., 8] -> c0, c1 of shape [..., 2(m/r), 2(reim)]: the c-th column
    (Re, Im) of phi(v).  phi entries: A=P00=(v0+v4)+i(v3+v7),
    B=P01=(v1-v5)+i(v6-v2), C=P10=(v1+v5)+i(v6+v2), D=P11=(v0-v4)+i(v7-v3)."""
    c0 = np.empty(v.shape[:-1] + (2, 2), dtype=v.dtype)
    c1 = np.empty_like(c0)
    v0, v1, v2, v3, v4, v5, v6, v7 = (v[..., a] for a in range(8))
    c0[..., 0, 0] = v0 + v4   # Re A
    c0[..., 0, 1] = v3 + v7   # Im A
    c0[..., 1, 0] = v1 + v5   # Re C
    c0[..., 1, 1] = v6 + v2   # Im C
    c1[..., 0, 0] = v1 - v5   # Re B
    c1[..., 0, 1] = v6 - v2   # Im B
    c1[..., 1, 0] = v0 - v4   # Re D
    c1[..., 1, 1] = v7 - v3   # Im D
    return c0, c1


def _np_bf16():
    return mybir.dt.np(mybir.dt.bfloat16)


def _prep_w(weight):
    """weight [COUT, CIN, 8] -> [3, 512, 512] planes [R, I, R+I] of
    phi(W)[r,m], rows (i,m), cols r-major (col = r*256 + o), 0.5 folded."""
    w = weight.astype(np.float32)
    cw0, cw1 = _pauli_parts(w)    # cw_m[o, i, r, (re,im)] = phi(W[o,i])[r,m]
    R = np.empty((CIN, 2, 2, COUT), np.float32)   # [(i,m),(r,o)]
    I = np.empty_like(R)
    for m, cm in ((0, cw0), (1, cw1)):
        for r in range(2):
            R[:, m, r, :] = 0.5 * cm[:, :, r, 0].T
            I[:, m, r, :] = 0.5 * cm[:, :, r, 1].T
    Rm = R.reshape(KP, 512)
    Im_ = I.reshape(KP, 512)
    return np.ascontiguousarray(
        np.stack([Rm, Im_], axis=0)).astype(_np_bf16())


def _prep_x(x):
    """x [B, CIN, 8] -> per-core arrays [N_CORES][BT, 128, XKT*128] bf16 for
    c in {0,1}: panels [XRe | XIm | XRe+XIm], device layout [bt, p, kk, b]
    with kappa = k*128 + p, col = kk*128 + b."""
    xf = x.astype(np.float32)
    c0, c1 = _pauli_parts(xf)          # [B, CIN, m, reim]
    outs = []
    for arr in (c0, c1):
        re = arr[..., 0].reshape(B, KP)          # kappa = i*2+m
        im = arr[..., 1].reshape(B, KP)
        panels = np.concatenate([re, im], axis=1)            # col = kk*128+p
        a = panels.reshape(N_CORES, BT, 128, XKT, 128)  # [core, bt, b, kk, p]
        a = a.transpose(0, 1, 4, 3, 2)                  # [core, bt, p, kk, b]
        outs.append(np.ascontiguousarray(
            a.reshape(N_CORES, BT, 128, XKT * 128)).astype(_np_bf16()))
    return outs


def kernel(x, weight, bias, cayley):
    assert x.shape == (B, CIN, NB) and weight.shape == (COUT, CIN, NB)
    if "nc" not in _cached:
        _cached["nc"] = _build_nc()
    nc = _cached["nc"]

    xp = _prep_x(np.asarray(x))
    wp = _prep_w(np.asarray(weight))
    in_maps = [{"xt0": xp[0][c], "xt1": xp[1][c], "wt": wp}
               for c in range(N_CORES)]
    res = run_bass_kernel_spmd(nc, in_maps, core_ids=list(range(N_CORES)))
    out = np.concatenate(
        [np.asarray(res.results[c]["out"]).astype(np.float32)
         for c in range(N_CORES)], axis=0)
    # cols are blade-major (blade*256 + o) -> [B, COUT, NB]
    out = out.reshape(B, NB, COUT).transpose(0, 2, 1)
    out = out + np.asarray(bias, np.float32)[None]
    return np.ascontiguousarray(out.astype(np.float32))


# revision 43
# speedup vs baseline: 1.3499x; 1.0028x over previous
"""CliffordLinear (Cl(3,0)) Trainium2 kernel.

Math: Cl(3,0) is isomorphic to the 2x2 complex matrices via the Pauli
representation phi.  The reference's per-channel Clifford contraction maps to
one complex matrix product  phi(Out)[:,c] = phi(W) @ phi(X)[:,c]  per output
column c in {0,1}, i.e. for each c the real [b x 512] panels XRe, XIm against
the real 512x512 planes R, I of phi(W):

    Re = XRe@R - XIm@I          Im = XRe@I + XIm@R

Gauss's 3-multiplication trick computes both from three products

    G1 = XRe@R   G2 = XIm@I   G3 = (XRe+XIm)@(R+I)
    Re = G1 - G2                Im = G3 - G1 - G2

which is 12 accumulation k-tiles per (batch-tile, c) instead of 16 -- a 25%
Tensor-engine saving (12.9G real MACs vs 17.2G for the 4-mult scheme, vs
34.4G naive blades).  The XRe+XIm panel and R+I plane are precomputed on the
host and shipped, so the trick costs no device arithmetic on the input side.

All operands move as bf16 (half the HBM traffic of fp32; matmuls run at the
same PE rate as fp32r).  Per (bt, c): PE accumulates G1/G2/G3 into three
one-bank PSUM tiles; ScalarE evicts each to bf16 SBUF; DVE does the Gauss
recombine and the inverse-Pauli blade butterfly entirely in packed bf16
(2x DVE rate), writing a blade-major stage tile that DMAs out as bf16.
Weight columns are r-major (col = r*256 + o) so every butterfly operand is
contiguous; the host unshuffles the blade-major output.

Sharding: data-parallel over batch (1024 rows/core); weights replicated.
Per-core HBM: 6.3 MB x + 1.6 MB w in, 4.2 MB out (~33 us at 360 B/ns),
under the ~41 us PE floor.
"""

import sys

sys.path.insert(0, "/opt/trn_rl_repo")

import numpy as np

import concourse.bass as bass  # noqa: F401  (registers lowerings)
import concourse.mybir as mybir
import concourse.tile as tile
from concourse import bacc
from concourse.bass_utils import run_bass_kernel_spmd

N_CORES = 8
B, CIN, COUT, NB = 8192, 256, 256, 8
BS = B // N_CORES          # 1024 batch rows per core
BT = BS // 128             # 8 b-tiles
KP = 2 * CIN               # 512 contraction rows per panel (i,m)
PKT = KP // 128            # 4 k-tiles per panel
XKT = 2 * PKT              # 8 x k-tiles per c (Re | Im; Sum panel is
                           # recomputed on-device, saving 1/3 of x HBM)
OUTW = COUT * NB           # 2048 output cols (blade-major: col = blade*256+o)

_cached = {}

N_WARM = 13        # PE p-state ramp matmuls before real data lands
PS_DB0 = 2         # ring depth for PSUM tag g00 (first group of each bt)
PS_DB1 = 1         # ring depth for PSUM tag g10 (g3b needs the 8th bank)
TIM_C0_POOL = False  # gpsimd tIm_c0 tested slower (Pool ALU 0.42 eff)


def _rw_ap(base, off, dims):
    """Clone `base` keeping its leading (partition/row) dim, replacing the
    free dims with `dims` [(step, num), ...] and adding `off` elements."""
    a = base.copy()
    part = a.ap.to_list()[0]
    v = a.ap
    v.clear()
    v.extend([tuple(part)] + [tuple(d) for d in dims])
    a.offset = a.offset + off
    return a


def _build_nc():
    bf = mybir.dt.bfloat16
    f32 = mybir.dt.float32
    nc = bacc.Bacc("TRN2", target_bir_lowering=False, debug=False,
                   num_devices=N_CORES)
    # x panels per phi-column c: [bt, p, kk*128 + b] with contraction row
    # kappa = k*128 + p inside panel kk//4 (0:XRe, 1:XIm, 2:XRe+XIm)
    xt = [nc.dram_tensor(f"xt{c}", [BT, 128, XKT * 128], bf,
                         kind="ExternalInput") for c in range(2)]
    # weight planes [R, I]: rows kappa=(i,m), cols r-major r*256+o.
    # The Gauss R+I plane is summed on-device (saves startup HBM traffic).
    wt = nc.dram_tensor("wt", [2, KP, 512], bf, kind="ExternalInput")
    out = nc.dram_tensor("out", [BS, OUTW], bf, kind="ExternalOutput")

    with tile.TileContext(nc) as tc:
        with tc.tile_pool(name="wpool", bufs=1) as wpool, \
             tc.tile_pool(name="xpool", bufs=3) as xpool, \
             tc.tile_pool(name="epool", bufs=2) as epool, \
             tc.tile_pool(name="pspool", bufs=1, space="PSUM") as pspool:
            # PE warmup on zeros: ramps the PE p-state during the initial DMA
            # wait so real matmuls start at full clock.
            warm_in = wpool.tile([128, 384], bf, tag="warm_in")
            nc.vector.memset(warm_in[:].bitcast(mybir.dt.uint32), 0)
            # warm_ps shares the g00 ring (retired after warmup, so the
            # spare PSUM banks go to double-buffering g00/g10 instead)
            warm_ps = pspool.tile([128, 512], f32, tag="g00", bufs=PS_DB0)
            for _ in range(N_WARM):
                nc.tensor.matmul(warm_ps[:, 0:256], warm_in[:, :128],
                                 warm_in[:, 128:384], start=True, stop=True)

            # Startup: interleave weight k-tiles with bt0's x panels in PE
            # consumption order (bt0 runs G-major: G1 both c, then G2, G3).
            # The R+I plane never moves over HBM -- DVE sums it from the R
            # and I tiles while the PE chews on G1/G2.
            w_t = [[None] * PKT for _ in range(3)]
            x0_p = [[None] * 3 for _ in range(2)]   # [c][panel] tiles for bt0

            def _w_kgrp(p, k0, nk):
                # one DMA covering k-tiles [k0, k0+nk) of plane p into
                # separate 512-col views of one tile: src rows kappa =
                # k*128 + p_low -> dest [p_low, k, col]
                t = wpool.tile([128, nk * 512], bf, tag=f"w{p}g{k0}",
                               name=f"w{p}g{k0}")
                for k in range(nk):
                    w_t[p][k0 + k] = t[:, k * 512:(k + 1) * 512]
                src = wt[p].copy()
                spart = tuple(src.ap.to_list()[0])
                v = src.ap
                v.clear()
                v.extend([(spart[0], 128), (spart[0] * 128, nk), (1, 512)])
                src.offset = src.offset + k0 * 128 * spart[0]
                nc.sync.dma_start(_rw_ap(t[:], 0, [(512, nk), (1, 512)]), src)

            def wrhs(p, k):
                return w_t[p][k]

            def _x0_dma(c, p):
                # bt0's x panels ride the gpsimd SWDGE queue: Pool generates
                # descriptors in parallel with HWDGE, halving the startup
                # per-DMA overhead serialization.
                x0_p[c][p] = xpool.tile([128, 512], bf, tag=f"x0{c}{p}",
                                        bufs=1, name=f"x0{c}{p}")
                nc.gpsimd.dma_start(x0_p[c][p][:],
                                    xt[c][0][:, p * 512:(p + 1) * 512])

            _x0_dma(0, 0)
            _w_kgrp(0, 0, 1)
            _x0_dma(1, 0)
            _w_kgrp(0, 1, 3)
            _x0_dma(0, 1)
            for k in range(PKT):
                _w_kgrp(1, k, 1)
            _x0_dma(1, 1)

            def _xsum(c, src_re, src_im):
                # on-device XSum panel for the G3 product
                xs = xpool.tile([128, 512], bf, tag=f"xs{c}", bufs=2,
                                name=f"xs{c}")
                nc.vector.tensor_add(xs[:], src_re, src_im)
                return xs

            xs0_c0 = _xsum(0, x0_p[0][0][:], x0_p[0][1][:])
            for k in range(PKT):
                w_t[2][k] = wpool.tile([128, 512], bf, tag=f"w2{k}",
                                       name=f"w2{k}")
                nc.vector.tensor_add(w_t[2][k][:], w_t[0][k], w_t[1][k])
            xs0_c1 = _xsum(1, x0_p[1][0][:], x0_p[1][1][:])

            def _mk_xlhs(panels, xs):
                # panels[c][p] for p in {0,1} are 512-col APs; xs[c] the sum
                def xlhs(c, p, k):
                    if p == 2:
                        return xs[c][:, k * 128:(k + 1) * 128]
                    return panels[c][p][:, k * 128:(k + 1) * 128]
                return xlhs

            cur_xlhs = _mk_xlhs(x0_p, [xs0_c0, xs0_c1])

            for bt in range(BT):
                # Prefetch bt+1's x one full window ahead: DMA + the
                # on-device XSum add, so neither is ever on the PE's path.
                if bt + 1 < BT:
                    nxt = []
                    for c in range(2):
                        t = xpool.tile([128, XKT * 128], bf, tag=f"x{c}",
                                       name=f"x{c}")
                        nc.sync.dma_start(t[:], xt[c][bt + 1])
                        nxt.append(t)
                    panels = [[t[:, 0:512], t[:, 512:1024]] for t in nxt]
                    xs = [_xsum(c, panels[c][0], panels[c][1])
                          for c in range(2)]
                    next_xlhs = _mk_xlhs(panels, xs)

                xlhs = cur_xlhs
                if bt + 1 < BT:
                    cur_xlhs = next_xlhs

                last = bt == BT - 1
                Gs = [[None] * 3, [None] * 3]

                def emit_mm(c, p, half=None, Gs=Gs, xlhs=xlhs):
                    bufs = (PS_DB0 if c == 0 else PS_DB1) if p == 0 else 1
                    g = pspool.tile([128, 512], f32, tag=f"g{c}{p}",
                                    name=f"g{c}{p}", bufs=bufs)
                    for k in range(PKT):
                        rhs = wrhs(p, k) if half is None else \
                            wrhs(p, k)[:, 256 * half:256 * (half + 1)]
                        o = g[:] if half is None else g[:, 0:256]
                        nc.tensor.matmul(o, xlhs(c, p, k), rhs,
                                         start=(k == 0), stop=(k == PKT - 1))
                    Gs[c][p] = g

                g3b = None
                if bt == 0 or last:
                    # G-major: bt0 matches the startup DMA arrival order; the
                    # last bt wants G1/G2 stopped early so the Re-blade
                    # eviction and stores run under the G3 matmuls.
                    for p in range(2):
                        for c in range(2):
                            emit_mm(c, p)
                    emit_mm(0, 2)
                    if last:
                        # G3c1 as two half-column PSUM groups: the first
                        # half's Im eviction+store pipelines under the
                        # second half's matmuls.
                        emit_mm(1, 2, half=0)
                        g3b = pspool.tile([128, 512], f32, tag="g3b")
                        for k in range(PKT):
                            nc.tensor.matmul(g3b[:, 0:256], xlhs(1, 2, k),
                                             wrhs(2, k)[:, 256:512],
                                             start=(k == 0), stop=(k == PKT - 1))
                    else:
                        emit_mm(1, 2)
                else:
                    for c in range(2):
                        for p in range(3):
                            emit_mm(c, p)

                # Eviction.  ScalarE copies PSUM -> bf16 SBUF (DVE reads at
                # most one PSUM operand, and all-bf16 doubles the DVE rate);
                # DVE does the Gauss recombine into t = [Re 512 | Im 512]
                # (r-major halves: A/C = r0/r1 of c0, B/D = r0/r1 of c1) and
                # the inverse-Pauli butterfly into the blade-major stage:
                #   x0 = ReA+ReD  x4 = ReA-ReD  x7 = ImA+ImD  x3 = ImA-ImD
                #   x1 = ReC+ReB  x5 = ReC-ReB  x6 = ImC+ImB  x2 = ImC-ImB
                add, sub = nc.vector.tensor_add, nc.vector.tensor_sub
                inner = (1, 256)
                stage = epool.tile([128, OUTW], bf, tag="stage")
                orows = out[bt * 128:(bt + 1) * 128, 0:OUTW]
                # ACT copies in matmul-stop order so no copy head-of-line
                # blocks an already-stopped G behind it on the in-order ACT
                # engine (stops are G-major on bt0/last, c-major otherwise).
                t_c, u_c = [], []
                gs_c = [[None] * 3, [None] * 3]
                np_copy = 2 if last else 3
                order = [(c, p) for p in range(np_copy) for c in range(2)] \
                    if (bt == 0 or last) else \
                    [(c, p) for c in range(2) for p in range(np_copy)]
                for c, p in order:
                    s = epool.tile([128, 512], bf, tag=f"gs{c}{p}",
                                   name=f"gs{c}{p}")
                    nc.scalar.copy(s[:], Gs[c][p][:])
                    gs_c[c][p] = s
                for c in range(2):
                    gs = gs_c[c]
                    t = epool.tile([128, 1024], bf, tag=f"t{c}", name=f"t{c}")
                    u = epool.tile([128, 512], bf, tag=f"u{c}", name=f"u{c}")
                    nc.vector.tensor_sub(t[:, 0:512], gs[0][:], gs[1][:])
                    nc.vector.tensor_add(u[:], gs[0][:], gs[1][:])
                    if not last:
                        nc.vector.tensor_sub(t[:, 512:1024], gs[2][:], u[:])
                    t_c.append(t)
                    u_c.append(u)

                if not last:
                    # Dual-blade butterfly ops; j picks the Re/Im halves.
                    add(_rw_ap(stage[:], 0 * 256, [(1792, 2), inner]),
                        _rw_ap(t_c[0][:], 0, [(512, 2), inner]),
                        _rw_ap(t_c[1][:], 256, [(512, 2), inner]))
                    sub(_rw_ap(stage[:], 4 * 256, [(-256, 2), inner]),
                        _rw_ap(t_c[0][:], 0, [(512, 2), inner]),
                        _rw_ap(t_c[1][:], 256, [(512, 2), inner]))
                    add(_rw_ap(stage[:], 1 * 256, [(1280, 2), inner]),
                        _rw_ap(t_c[0][:], 256, [(512, 2), inner]),
                        _rw_ap(t_c[1][:], 0, [(512, 2), inner]))
                    sub(_rw_ap(stage[:], 5 * 256, [(-768, 2), inner]),
                        _rw_ap(t_c[0][:], 256, [(512, 2), inner]),
                        _rw_ap(t_c[1][:], 0, [(512, 2), inner]))
                    # Steady stores ride gpsimd's SWDGE queue: the sem wait
                    # parks on the otherwise-idle Pool SEQ, so the SP load
                    # queue never stalls behind a store.
                    nc.gpsimd.dma_start(orows, stage[:])
                else:
                    # Re/Im-phased tail: Re blades (j duals (x0,x1), (x4,x5))
                    # need only G1/G2 -- they evict and store while the G3
                    # matmuls still run.  Only the Im blades wait on G3.
                    add(_rw_ap(stage[:], 0, [(256, 2), inner]),
                        _rw_ap(t_c[0][:], 0, [(256, 2), inner]),
                        _rw_ap(t_c[1][:], 256, [(-256, 2), inner]))
                    sub(_rw_ap(stage[:], 1024, [(256, 2), inner]),
                        _rw_ap(t_c[0][:], 0, [(256, 2), inner]),
                        _rw_ap(t_c[1][:], 256, [(-256, 2), inner]))
                    nc.gpsimd.dma_start(
                        _rw_ap(orows, 0, [(1024, 2), (1, 512)]),
                        _rw_ap(stage[:], 0, [(1024, 2), (1, 512)]))
                    # Im phase.  c0 full-width (runs under G3c1's matmuls);
                    # c1 in pipelined halves a (cols 0:256 = ImB) and b
                    # (256:512 = ImD, the g3b bank).  Blade singles:
                    #   x2 = ImC-ImB   x6 = ImC+ImB   (a half)
                    #   x3 = ImA-ImD   x7 = ImA+ImD   (b half)
                    s0 = epool.tile([128, 512], bf, tag="gs02", name="gs02")
                    nc.scalar.copy(s0[:], Gs[0][2][:])
                    sa = epool.tile([128, 512], bf, tag="gs12", name="gs12")
                    nc.scalar.copy(sa[:, 0:256], Gs[1][2][:, 0:256])
                    nc.scalar.copy(sa[:, 256:512], g3b[:, 0:256])
                    tim0_eng = nc.gpsimd if TIM_C0_POOL else nc.vector
                    tim0_eng.tensor_sub(t_c[0][:, 512:1024], s0[:], u_c[0][:])
                    nc.vector.tensor_sub(t_c[1][:, 512:768], sa[:, 0:256],
                                         u_c[1][:, 0:256])
                    sub(stage[:, 512:768], t_c[0][:, 768:1024],
                        t_c[1][:, 512:768])
                    add(stage[:, 1536:1792], t_c[0][:, 768:1024],
                        t_c[1][:, 512:768])
                    nc.scalar.dma_start(
                        _rw_ap(orows, 512, [(1024, 2), inner]),
                        _rw_ap(stage[:], 512, [(1024, 2), inner]))
                    nc.vector.tensor_sub(t_c[1][:, 768:1024], sa[:, 256:512],
                                         u_c[1][:, 256:512])
                    sub(stage[:, 768:1024], t_c[0][:, 512:768],
                        t_c[1][:, 768:1024])
                    add(stage[:, 1792:2048], t_c[0][:, 512:768],
                        t_c[1][:, 768:1024])
                    nc.sync.dma_start(
                        _rw_ap(orows, 768, [(1024, 2), inner]),
                        _rw_ap(stage[:], 768, [(1024, 2), inner]))
    nc.finalize()
    return nc


def _pauli_parts(v):
    """v[..., 8] -> c0, c1 of shape [..., 2(m/r), 2(reim)]: the c-th column
    (Re, Im) of phi(v).  phi entries: A=P00=(v0+v4)+i(v3+v7),
    B=P01=(v1-v5)+i(v6-v2), C=P10=(v1+v5)+i(v6+v2), D=P11=(v0-v4)+i(v7-v3)."""
    c0 = np.empty(v.shape[:-1] + (2, 2), dtype=v.dtype)
    c1 = np.empty_like(c0)
    v0, v1, v2, v3, v4, v5, v6, v7 = (v[..., a] for a in range(8))
    c0[..., 0, 0] = v0 + v4   # Re A
    c0[..., 0, 1] = v3 + v7   # Im A
    c0[..., 1, 0] = v1 + v5   # Re C
    c0[..., 1, 1] = v6 + v2   # Im C
    c1[..., 0, 0] = v1 - v5   # Re B
    c1[..., 0, 1] = v6 - v2   # Im B
    c1[..., 1, 0] = v0 - v4   # Re D
    c1[..., 1, 1] = v7 - v3   # Im D
    return c0, c1


def _np_bf16():
    return mybir.dt.np(mybir.dt.bfloat16)


def _prep_w(weight):
    """weight [COUT, CIN, 8] -> [3, 512, 512] planes [R, I, R+I] of
    phi(W)[r,m], rows (i,m), cols r-major (col = r*256 + o), 0.5 folded."""
    w = weight.astype(np.float32)
    cw0, cw1 = _pauli_parts(w)    # cw_m[o, i, r, (re,im)] = phi(W[o,i])[r,m]
    R = np.empty((CIN, 2, 2, COUT), np.float32)   # [(i,m),(r,o)]
    I = np.empty_like(R)
    for m, cm in ((0, cw0), (1, cw1)):
        for r in range(2):
            R[:, m, r, :] = 0.5 * cm[:, :, r, 0].T
            I[:, m, r, :] = 0.5 * cm[:, :, r, 1].T
    Rm = R.reshape(KP, 512)
    Im_ = I.reshape(KP, 512)
    return np.ascontiguousarray(
        np.stack([Rm, Im_], axis=0)).astype(_np_bf16())


def _prep_x(x):
    """x [B, CIN, 8] -> per-core arrays [N_CORES][BT, 128, XKT*128] bf16 for
    c in {0,1}: panels [XRe | XIm | XRe+XIm], device layout [bt, p, kk, b]
    with kappa = k*128 + p, col = kk*128 + b."""
    xf = x.astype(np.float32)
    c0, c1 = _pauli_parts(xf)          # [B, CIN, m, reim]
    outs = []
    for arr in (c0, c1):
        re = arr[..., 0].reshape(B, KP)          # kappa = i*2+m
        im = arr[..., 1].reshape(B, KP)
        panels = np.concatenate([re, im], axis=1)            # col = kk*128+p
        a = panels.reshape(N_CORES, BT, 128, XKT, 128)  # [core, bt, b, kk, p]
        a = a.transpose(0, 1, 4, 3, 2)                  # [core, bt, p, kk, b]
        outs.append(np.ascontiguousarray(
            a.reshape(N_CORES, BT, 128, XKT * 128)).astype(_np_bf16()))
    return outs


def kernel(x, weight, bias, cayley):
    assert x.shape == (B, CIN, NB) and weight.shape == (COUT, CIN, NB)
    if "nc" not in _cached:
        _cached["nc"] = _build_nc()
    nc = _cached["nc"]

    xp = _prep_x(np.asarray(x))
    wp = _prep_w(np.asarray(weight))
    in_maps = [{"xt0": xp[0][c], "xt1": xp[1][c], "wt": wp}
               for c in range(N_CORES)]
    res = run_bass_kernel_spmd(nc, in_maps, core_ids=list(range(N_CORES)))
    out = np.concatenate(
        [np.asarray(res.results[c]["out"]).astype(np.float32)
         for c in range(N_CORES)], axis=0)
    # cols are blade-major (blade*256 + o) -> [B, COUT, NB]
    out = out.reshape(B, NB, COUT).transpose(0, 2, 1)
    out = out + np.asarray(bias, np.float32)[None]
    return np.ascontiguousarray(out.astype(np.float32))
